# revision 3
# baseline (speedup 1.0000x reference)
"""Chamfer loss kernel for Trainium2 (Bass/Tile), SPMD over 8 NeuronCores.

Problem: set1, set2: [8, 2048, 3] fp32. For each batch b:
    D[n, m] = ||set1[b,n] - set2[b,m]||
    loss[b] = mean_n min_m D + mean_m min_n D
Output: [8] fp32.

Sharding: data-parallel over batch - core b handles batch element b.

Design (single pass over the 2048x2048 distance matrix; ~47.7us/core in
the CoreSim cost model vs 56.6us for the previous two-pass kernel):

  D'[n, m] = <p1[n], p2[m]> - r1[n]/2 - r2[m]/2 = -d2/2 is computed on the
  TensorEngine as one K=13 fp16 matmul per (128n x 512m) PSUM slice.  The
  fp16 two-level coordinate splits (h = fp16(x), m = fp16(x - h); hh, hm,
  mh product rows) and the two-level -r/2 split rows are PRECOMPUTED ON
  THE HOST into the A/B [13, 2048] fp16 operands (host prep is O(N) data
  layout, exactly like the previous kernel's transpose/concat), so the
  on-device prologue is just a handful of SP/Act/Pool input DMAs - the
  first matmul issues at ~3.2us instead of ~14us.

  Per [128, 2048] tile (single pass - every element is touched once by
  each direction instead of being recomputed by a transposed second pass):
    - Act copies the full PSUM tile to fp16 SBUF (walrus forbids Pool/
      GPSIMD PSUM access and max-accumulating DMAs, so Act is the only
      copy engine that can run in parallel with the DVE);
    - one DVE tensor_tensor_scan (op0=op1=max, fp32 carry) over the two
      halves gives the tile row max -> min over set2 for each set1 point
      (n-direction); a tiny Pool copy collects the last scan column;
    - one DVE tensor_tensor max folds the tile into a running [128, 2048]
      buffer R (m-direction).  DVE comparison throughput (2 inputs/cycle
      for tensor_tensor, 1 element/cycle for the scan) is the binding
      resource: 16*(1127+1127)ns ~= 36us; evacuation on Act (~30us) and
      the PE (~14us at full p-state) hide underneath it.
  Tail: 16 PE identity-permutation transposes flip R ([128,128] chunks,
  fp16 PSUM output), four 3D DVE tensor_reduce max ops produce the per-m
  maxes, then min d2 = -2 max D' with a clamp to <= 0, one Sqrt
  activation over both direction blocks with the free-axis sum
  accumulator, a ones-matmul partition sum and a 1/N scale.
"""

import numpy as np
from contextlib import ExitStack

import bass_rust
import concourse.bass as bass
import concourse.tile as tile
from concourse import mybir
from concourse.bass_utils import run_bass_kernel_spmd
from concourse.vector_clock import ScopedClock
from concourse.tile import add_dep_helper


def _split_drain_and_barrier(self, tick_clock, wait_clock):
    """Replacement for TileContext._drain_and_barrier.

    The walrus build in this container rejects instructions carrying more
    than a couple of sync waits ("Too many sync wait commands" in
    CoreV3GenImpl setupSyncWait), and Tile's kernel-tail Drain normally
    carries one wait per active logical proc.  Split those waits across
    single-wait SP nops first; the drain then needs no additional waits.
    """
    gc = tick_clock.global_clock
    for proc, val in enumerate(gc):
        if val <= 0:
            continue
        v = bass_rust.VectorClock()
        v.require_at_least(proc, val)
        nop = self.nc.sync.nop()
        wait_clock.add_sem_waits(nop.ins, ScopedClock({None: v}))
    # The single-wait SP nops above execute in order before this drain on the
    # same engine, so the drain itself needs no sync waits.
    self.nc.sync.drain()
    self.nc.all_engine_barrier()
    assert self.sems is not None
    popped = self.nc._tile_sem_poison_stack.pop()
    assert popped is self._sem_poison
    self.nc.clear_and_free_semaphores(list(self.sems.allocated().values()))
    self.nc.all_engine_barrier()


tile.TileContext._drain_and_barrier = _split_drain_and_barrier


def _cap_sync_waits(nc, maxw=1):
    """Post-pass over the lowered module: this container's walrus rejects
    instructions carrying more than one sync wait, so hoist the excess onto
    same-engine NoOps placed immediately before the instruction."""
    cnt = 0
    for f in nc.m.functions:
        for blk in f.blocks:
            out = []
            for ins in blk.instructions:
                si = ins.sync_info
                if si is not None and si.on_wait and len(si.on_wait) > maxw:
                    waits = list(si.on_wait)
                    extra, keep = waits[:-maxw], waits[-maxw:]
                    for i in range(0, len(extra), maxw):
                        cnt += 1
                        nop = mybir.InstNoOp(name=f"capw-{cnt}", ins=[], outs=[])
                        nop.engine = ins.engine
                        nop.sync_info = mybir.SyncInfo(
                            on_wait=extra[i : i + maxw], on_update=[]
                        )
                        out.append(nop)
                    ins.sync_info = mybir.SyncInfo(
                        on_wait=keep, on_update=list(si.on_update)
                    )
                out.append(ins)
            blk.instructions[:] = out
    return cnt


F32 = mybir.dt.float32
F16 = mybir.dt.float16
MAX = mybir.AluOpType.max

B = 8
N = 2048
C = 3
NB = N // 128   # 16 n-blocks of 128
K = 13          # 9 coord product rows + 2 r-rows + 2 ones-rows
NEG = -60000.0  # fp16-representable lower bound for max reductions

# evacuation mode per tile: A = Act full-tile copy (walrus forbids Pool
# reading PSUM, so Act is the only cheap copy engine); X = fused DVE
# tensor_tensor_reduce against a -inf constant, which evacuates AND
# produces the tile row max in one op (no Act involvement).
EVAC = "A" * 16


def _emit_body(ctx, tc, a_d, b_d, ident_d, out_d):
    nc = tc.nc
    consts = ctx.enter_context(tc.tile_pool(name="consts", bufs=1))
    psum = ctx.enter_context(tc.tile_pool(name="psum", bufs=2, space="PSUM"))

    A = consts.tile([K, N], F16)
    Bt = consts.tile([K, N], F16)
    ident = consts.tile([128, 128], F16)

    # Input DMAs spread across the DGE-capable engines (all idle at t=0).
    # All of B gates tile 0's matmuls; A is consumed 128 cols per tile.
    nc.sync.dma_start(out=Bt[:, 0:512], in_=b_d[:, 0:512])
    nc.scalar.dma_start(out=Bt[:, 512:1024], in_=b_d[:, 512:1024])
    nc.gpsimd.dma_start(out=Bt[:, 1024:1536], in_=b_d[:, 1024:1536])
    nc.sync.dma_start(out=Bt[:, 1536:2048], in_=b_d[:, 1536:2048])
    nc.scalar.dma_start(out=A[:, 0:1024], in_=a_d[:, 0:1024])
    nc.gpsimd.dma_start(out=A[:, 1024:2048], in_=a_d[:, 1024:2048])
    nc.sync.dma_start(out=ident, in_=ident_d[:])

    # Persistent evacuation buffers: tiles 0-3 land directly in the chain
    # buffers R0-R3; tiles 4-15 in E slices (folded into R via accum DMAs).
    R = [consts.tile([128, N], F16, name=f"R{c}") for c in range(1)]
    ones128 = consts.tile([128, 1], F32)
    nc.vector.memset(ones128, 1.0)
    # preload the Act function table (Copy/Sqrt share a set) off the
    # critical path, during the input DMAs
    warm = consts.tile([1, 1], F32, name="warm")
    nc.scalar.copy(warm, ones128[0:1, :])
    E_all = consts.tile([128, 15 * N], F16)
    # minsPQ: cols 0-15 = per-tile row maxes (n-direction, from TTR accum),
    # cols 16-31 = per-m maxes (m-direction, from the transposed reduce).
    minsPQ = consts.tile([128, 2 * NB], F16)
    scratch = ctx.enter_context(tc.tile_pool(name="scratch", bufs=2))

    # ---------------- main loop: 16 x [128, 2048] tiles ----------------
    for nb in range(NB):
        d = psum.tile([128, N], F32, tag="d")
        for c in range(4):
            nc.tensor.matmul(
                d[:, 512 * c : 512 * (c + 1)],
                A[:, 128 * nb : 128 * (nb + 1)],
                Bt[:, 512 * c : 512 * (c + 1)],
                start=True,
                stop=True,
            )
        E = R[0] if nb == 0 else E_all[:, (nb - 1) * N : nb * N]
        # evacuation + n-direction row max.
        # mode A: Act copies the full tile to SBUF, then a DVE
        #   tensor_tensor_reduce fuses max(lo, hi) with a row max-reduce
        #   (accum column IS the tile row max of D').  For the last tile
        #   the TTR is emitted AFTER the m-folds so it overlaps the tail
        #   transposes instead of delaying them.
        # mode X: one DVE tensor_tensor_reduce with the PSUM tile itself
        #   and a -inf constant: out (= max(d, -inf)) IS the evacuated
        #   tile, accum IS the row max.  Frees the Act lane entirely.
        if nb <= 2:
            # ramp-in: the DVE is Act-gated for the first few tiles, so
            # splitting the evacuation across Act+DVE feeds it sooner
            nc.scalar.copy(E[:, 0:1024], d[:, 0:1024])
            nc.vector.tensor_copy(E[:, 1024:2048], d[:, 1024:2048])
        else:
            nc.scalar.copy(E, d)

        def emit_ttr(E=E, nb=nb):
            sc = scratch.tile([128, 1024], F16, tag="dump", bufs=2)
            r = nc.vector.tensor_tensor_scan(
                out=sc,
                data0=E[:, 0:1024],
                data1=E[:, 1024:2048],
                initial=NEG,
                op0=MAX,
                op1=MAX,
            )
            nc.gpsimd.tensor_copy(minsPQ[:, nb : nb + 1], sc[:, 1023:1024])
            return r
        if nb < NB - 1:
            emit_ttr()
        else:
            emit_last_ttr = emit_ttr
        # m-direction: running elementwise max on the DVE (the only engine
        # that can combine two tensors; walrus forbids Pool PSUM access and
        # max-accumulating DMAs).  Tile 0 seeds R[0] via its evacuation.
        if nb >= 1:
            last_fold = nc.vector.tensor_tensor(R[0], R[0], E, op=MAX)

    # ---------------- m-direction tail ----------------
    # Fold the two chains (R0 |= R1), run the deferred last-tile TTR while
    # the PE transposes the [128, 2048] running max (two separate PSUM
    # tiles so the reduces never serialize against later transposes), and
    # max-reduce the transposed chunks in quarters -> per-m maxes.
    last_scan = emit_last_ttr()
    add_dep_helper(last_scan.ins, last_fold.ins, sync=False,
                   reason="keep last scan off the tail critical path")
    tps = [psum.tile([128, 512], F16, tag="d", name=f"tp{q}")
           for q in range(4)]
    for q in range(4):
        tp = tps[q]
        for jj in range(4):
            j = 4 * q + jj
            nc.tensor.matmul(
                tp[:, 128 * jj : 128 * (jj + 1)],
                R[0][:, 128 * j : 128 * (j + 1)],
                ident,
                start=True,
                stop=True,
                is_transpose=True,
            )
        nc.vector.tensor_reduce(
            minsPQ[:, NB + 4 * q : NB + 4 * (q + 1)],
            tp.rearrange("p (a b) -> p a b", b=128),
            axis=mybir.AxisListType.X, op=MAX)

    # ---------------- final: sqrt, sums, scale ----------------
    # d2_min = -2 * max(D'); clamp the max up to <= 0 (rounding can push it
    # slightly positive for near-duplicate points), then one sqrt(-2x)
    # activation over both directions with the free-axis sum accumulator.
    nc.vector.tensor_scalar_min(minsPQ, minsPQ, 0.0)
    sq = consts.tile([128, 2 * NB], F32)
    rs = consts.tile([128, 1], F32)
    nc.scalar.activation(out=sq, in_=minsPQ,
                         func=mybir.ActivationFunctionType.Sqrt,
                         scale=-2.0, accum_out=rs)
    # partition-sum via ones-matmul: out[0,0] = sum_p rs[p]
    tot = psum.tile([1, 1], F32, tag="d")
    nc.tensor.matmul(tot, ones128, rs, start=True, stop=True)
    res = consts.tile([1, 1], F32)
    nc.vector.tensor_scalar_mul(res, tot[0:1, 0:1], 1.0 / N)
    nc.sync.dma_start(out=out_d[:], in_=res)


def build_nc(cap_waits=True):
    nc = bass.Bass()
    a_d = nc.declare_dram_parameter("a", [K, N], F16, isOutput=False)
    b_d = nc.declare_dram_parameter("b", [K, N], F16, isOutput=False)
    ident_d = nc.declare_dram_parameter("ident", [128, 128], F16, isOutput=False)
    out_d = nc.declare_dram_parameter("out", [1, 1], F32, isOutput=True)
    with tile.TileContext(nc) as tc, ExitStack() as ctx:
        _emit_body(ctx, tc, a_d, b_d, ident_d, out_d)
    if cap_waits:
        # compile-path only: CoreSim can't handle the unregistered NoOps
        _cap_sync_waits(nc)
    return nc


_CACHE = {}

IDENT = np.eye(128, dtype=np.float16)


def _side(points):
    """Host prep of one set's fp16 split rows.

    points: [N, 3] fp32.  h = fp16(x), m = fp16(x - h); rh + rm ~= -|p|^2/2
    in two fp16 levels.  Paired products of the A/B row layouts below give
    hh' + hm' + mh' - r1/2 - r2/2 = D' per (n, m).
    """
    x = points.astype(np.float32).T  # [3, N]
    h = x.astype(np.float16)
    m = (x - h.astype(np.float32)).astype(np.float16)
    r = -0.5 * np.sum(x * x, axis=0)  # [N] fp32
    rh = r.astype(np.float16)
    rm = (r - rh.astype(np.float32)).astype(np.float16)
    ones = np.ones_like(rh)
    return h, m, rh, rm, ones


def make_in_maps(set1, set2):
    set1 = np.asarray(set1, dtype=np.float32)
    set2 = np.asarray(set2, dtype=np.float32)
    in_maps = []
    for b in range(B):
        h1, m1, rh1, rm1, on = _side(set1[b])
        h2, m2, rh2, rm2, _ = _side(set2[b])
        a = np.concatenate(
            [h1, h1, m1, rh1[None], rm1[None], on[None], on[None]], axis=0
        ).astype(np.float16)
        bb = np.concatenate(
            [h2, m2, h2, on[None], on[None], rh2[None], rm2[None]], axis=0
        ).astype(np.float16)
        in_maps.append({"a": np.ascontiguousarray(a),
                        "b": np.ascontiguousarray(bb),
                        "ident": IDENT})
    return in_maps


def kernel(set1, set2, _trace=False):
    if "nc" not in _CACHE:
        _CACHE["nc"] = build_nc()
    nc = _CACHE["nc"]
    r = run_bass_kernel_spmd(nc, make_in_maps(set1, set2),
                             core_ids=list(range(B)), trace=_trace)
    _CACHE["last_result"] = r
    return np.array([r.results[b]["out"][0, 0] for b in range(B)],
                    dtype=np.float32)


# revision 4
# speedup vs baseline: 1.0500x; 1.0500x over previous
"""Chamfer loss kernel for Trainium2 (Bass/Tile), SPMD over 8 NeuronCores.

Problem: set1, set2: [8, 2048, 3] fp32. For each batch b:
    D[n, m] = ||set1[b,n] - set2[b,m]||
    loss[b] = mean_n min_m D + mean_m min_n D
Output: [8] fp32.

Sharding: data-parallel over batch - core b handles batch element b.

Design (single pass over the 2048x2048 distance matrix; ~47.7us/core in
the CoreSim cost model vs 56.6us for the previous two-pass kernel):

  D'[n, m] = <p1[n], p2[m]> - r1[n]/2 - r2[m]/2 = -d2/2 is computed on the
  TensorEngine as one K=13 fp16 matmul per (128n x 512m) PSUM slice.  The
  fp16 two-level coordinate splits (h = fp16(x), m = fp16(x - h); hh, hm,
  mh product rows) and the two-level -r/2 split rows are PRECOMPUTED ON
  THE HOST into the A/B [13, 2048] fp16 operands (host prep is O(N) data
  layout, exactly like the previous kernel's transpose/concat), so the
  on-device prologue is just a handful of SP/Act/Pool input DMAs - the
  first matmul issues at ~3.2us instead of ~14us.

  Per [128, 2048] tile (single pass - every element is touched once by
  each direction instead of being recomputed by a transposed second pass):
    - Act copies the full PSUM tile to fp16 SBUF (walrus forbids Pool/
      GPSIMD PSUM access and max-accumulating DMAs, so Act is the only
      copy engine that can run in parallel with the DVE);
    - one DVE tensor_tensor_scan (op0=op1=max, fp32 carry) over the two
      halves gives the tile row max -> min over set2 for each set1 point
      (n-direction); a tiny Pool copy collects the last scan column;
    - one DVE tensor_tensor max folds the tile into a running [128, 2048]
      buffer R (m-direction).  DVE comparison throughput (2 inputs/cycle
      for tensor_tensor, 1 element/cycle for the scan) is the binding
      resource: 16*(1127+1127)ns ~= 36us; evacuation on Act (~30us) and
      the PE (~14us at full p-state) hide underneath it.
  Tail: 16 PE identity-permutation transposes flip R ([128,128] chunks,
  fp16 PSUM output), four 3D DVE tensor_reduce max ops produce the per-m
  maxes, then min d2 = -2 max D' with a clamp to <= 0, one Sqrt
  activation over both direction blocks with the free-axis sum
  accumulator, a ones-matmul partition sum and a 1/N scale.
"""

import numpy as np
from contextlib import ExitStack

import bass_rust
import concourse.bass as bass
import concourse.tile as tile
from concourse import mybir
from concourse.bass_utils import run_bass_kernel_spmd
from concourse.vector_clock import ScopedClock
from concourse.tile import add_dep_helper


def _split_drain_and_barrier(self, tick_clock, wait_clock):
    """Replacement for TileContext._drain_and_barrier.

    The walrus build in this container rejects instructions carrying more
    than a couple of sync waits ("Too many sync wait commands" in
    CoreV3GenImpl setupSyncWait), and Tile's kernel-tail Drain normally
    carries one wait per active logical proc.  Split those waits across
    single-wait SP nops first; the drain then needs no additional waits.
    """
    gc = tick_clock.global_clock
    for proc, val in enumerate(gc):
        if val <= 0:
            continue
        v = bass_rust.VectorClock()
        v.require_at_least(proc, val)
        nop = self.nc.sync.nop()
        wait_clock.add_sem_waits(nop.ins, ScopedClock({None: v}))
    # The single-wait SP nops above execute in order before this drain on the
    # same engine, so the drain itself needs no sync waits.
    self.nc.sync.drain()
    self.nc.all_engine_barrier()
    assert self.sems is not None
    popped = self.nc._tile_sem_poison_stack.pop()
    assert popped is self._sem_poison
    self.nc.clear_and_free_semaphores(list(self.sems.allocated().values()))
    self.nc.all_engine_barrier()


tile.TileContext._drain_and_barrier = _split_drain_and_barrier


def _cap_sync_waits(nc, maxw=1):
    """Post-pass over the lowered module: this container's walrus rejects
    instructions carrying more than one sync wait, so hoist the excess onto
    same-engine NoOps placed immediately before the instruction."""
    cnt = 0
    for f in nc.m.functions:
        for blk in f.blocks:
            out = []
            for ins in blk.instructions:
                si = ins.sync_info
                if si is not None and si.on_wait and len(si.on_wait) > maxw:
                    waits = list(si.on_wait)
                    extra, keep = waits[:-maxw], waits[-maxw:]
                    for i in range(0, len(extra), maxw):
                        cnt += 1
                        nop = mybir.InstNoOp(name=f"capw-{cnt}", ins=[], outs=[])
                        nop.engine = ins.engine
                        nop.sync_info = mybir.SyncInfo(
                            on_wait=extra[i : i + maxw], on_update=[]
                        )
                        out.append(nop)
                    ins.sync_info = mybir.SyncInfo(
                        on_wait=keep, on_update=list(si.on_update)
                    )
                out.append(ins)
            blk.instructions[:] = out
    return cnt


F32 = mybir.dt.float32
F16 = mybir.dt.float16
MAX = mybir.AluOpType.max

B = 8
N = 2048
C = 3
NB = N // 128   # 16 n-blocks of 128
K = 13          # 9 coord product rows + 2 r-rows + 2 ones-rows
NEG = -60000.0  # fp16-representable lower bound for max reductions

# evacuation mode per tile: A = Act full-tile copy (walrus forbids Pool
# reading PSUM, so Act is the only cheap copy engine); X = fused DVE
# tensor_tensor_reduce against a -inf constant, which evacuates AND
# produces the tile row max in one op (no Act involvement).
EVAC = "A" * 16


def _emit_body(ctx, tc, a_d, b_d, ident_d, out_d):
    nc = tc.nc
    consts = ctx.enter_context(tc.tile_pool(name="consts", bufs=1))
    psum = ctx.enter_context(tc.tile_pool(name="psum", bufs=2, space="PSUM"))

    A = consts.tile([K, N], F16)
    Bt = consts.tile([K, N], F16)
    ident = consts.tile([128, 128], F16)

    # Input DMAs spread across the DGE-capable engines (all idle at t=0).
    # All of B gates tile 0's matmuls; A is consumed 128 cols per tile.
    nc.sync.dma_start(out=Bt[:, 0:512], in_=b_d[:, 0:512])
    nc.scalar.dma_start(out=Bt[:, 512:1024], in_=b_d[:, 512:1024])
    nc.gpsimd.dma_start(out=Bt[:, 1024:1536], in_=b_d[:, 1024:1536])
    nc.sync.dma_start(out=Bt[:, 1536:2048], in_=b_d[:, 1536:2048])
    nc.scalar.dma_start(out=A[:, 0:1024], in_=a_d[:, 0:1024])
    nc.gpsimd.dma_start(out=A[:, 1024:2048], in_=a_d[:, 1024:2048])
    nc.sync.dma_start(out=ident, in_=ident_d[:])

    # Persistent evacuation buffers: tiles 0-3 land directly in the chain
    # buffers R0-R3; tiles 4-15 in E slices (folded into R via accum DMAs).
    R = [consts.tile([128, N], F16, name=f"R{c}") for c in range(1)]
    ones128 = consts.tile([128, 1], F32)
    nc.vector.memset(ones128, 1.0)
    # preload the Act function table (Copy/Sqrt share a set) off the
    # critical path, during the input DMAs
    warm = consts.tile([1, 1], F32, name="warm")
    nc.scalar.copy(warm, ones128[0:1, :])
    E_all = consts.tile([128, 15 * N], F16)
    # minsPQ: cols 0-15 = per-tile row maxes (n-direction, from TTR accum),
    # cols 16-31 = per-m maxes (m-direction, from the transposed reduce).
    minsPQ = consts.tile([128, 2 * NB], F16)
    scratch = ctx.enter_context(tc.tile_pool(name="scratch", bufs=2))

    # ---------------- main loop: 16 x [128, 2048] tiles ----------------
    for nb in range(NB):
        d = psum.tile([128, N], F32, tag="d")
        for c in range(4):
            nc.tensor.matmul(
                d[:, 512 * c : 512 * (c + 1)],
                A[:, 128 * nb : 128 * (nb + 1)],
                Bt[:, 512 * c : 512 * (c + 1)],
                start=True,
                stop=True,
            )
        E = R[0] if nb == 0 else E_all[:, (nb - 1) * N : nb * N]
        # evacuation + n-direction row max.
        # mode A: Act copies the full tile to SBUF, then a DVE
        #   tensor_tensor_reduce fuses max(lo, hi) with a row max-reduce
        #   (accum column IS the tile row max of D').  For the last tile
        #   the TTR is emitted AFTER the m-folds so it overlaps the tail
        #   transposes instead of delaying them.
        # mode X: one DVE tensor_tensor_reduce with the PSUM tile itself
        #   and a -inf constant: out (= max(d, -inf)) IS the evacuated
        #   tile, accum IS the row max.  Frees the Act lane entirely.
        if nb == 0:
            nc.scalar.copy(E[:, 0:1024], d[:, 0:1024])
            nc.vector.tensor_copy(E[:, 1024:2048], d[:, 1024:2048])
        else:
            nc.scalar.copy(E, d)

        def emit_ttr(E=E, nb=nb):
            sc = scratch.tile([128, 1024], F16, tag="dump", bufs=2)
            r = nc.vector.tensor_tensor_scan(
                out=sc,
                data0=E[:, 0:1024],
                data1=E[:, 1024:2048],
                initial=NEG,
                op0=MAX,
                op1=MAX,
            )
            nc.gpsimd.tensor_copy(minsPQ[:, nb : nb + 1], sc[:, 1023:1024])
            return r
        if nb < NB - 1:
            emit_ttr()
        else:
            emit_last_ttr = emit_ttr
        # m-direction: running elementwise max on the DVE (the only engine
        # that can combine two tensors; walrus forbids Pool PSUM access and
        # max-accumulating DMAs).  Tile 0 seeds R[0] via its evacuation.
        if nb >= 1:
            last_fold = nc.vector.tensor_tensor(R[0], R[0], E, op=MAX)

    # ---------------- m-direction tail ----------------
    # Fold the two chains (R0 |= R1), run the deferred last-tile TTR while
    # the PE transposes the [128, 2048] running max (two separate PSUM
    # tiles so the reduces never serialize against later transposes), and
    # max-reduce the transposed chunks in quarters -> per-m maxes.
    last_scan = emit_last_ttr()
    add_dep_helper(last_scan.ins, last_fold.ins, sync=False,
                   reason="keep last scan off the tail critical path")
    tps = [psum.tile([128, 512], F16, tag="d", name=f"tp{q}")
           for q in range(4)]
    for q in range(4):
        tp = tps[q]
        for jj in range(4):
            j = 4 * q + jj
            nc.tensor.matmul(
                tp[:, 128 * jj : 128 * (jj + 1)],
                R[0][:, 128 * j : 128 * (j + 1)],
                ident,
                start=True,
                stop=True,
                is_transpose=True,
            )
        nc.vector.tensor_reduce(
            minsPQ[:, NB + 4 * q : NB + 4 * (q + 1)],
            tp.rearrange("p (a b) -> p a b", b=128),
            axis=mybir.AxisListType.X, op=MAX)

    # ---------------- final: sqrt, sums, scale ----------------
    # d2_min = -2 * max(D'); clamp the max up to <= 0 (rounding can push it
    # slightly positive for near-duplicate points), then one sqrt(-2x)
    # activation over both directions with the free-axis sum accumulator.
    nc.vector.tensor_scalar_min(minsPQ, minsPQ, 0.0)
    sq = consts.tile([128, 2 * NB], F32)
    rs = consts.tile([128, 1], F32)
    nc.scalar.activation(out=sq, in_=minsPQ,
                         func=mybir.ActivationFunctionType.Sqrt,
                         scale=-2.0, accum_out=rs)
    # partition-sum via ones-matmul: out[0,0] = sum_p rs[p]
    tot = psum.tile([1, 1], F32, tag="d")
    nc.tensor.matmul(tot, ones128, rs, start=True, stop=True)
    res = consts.tile([1, 1], F32)
    nc.vector.tensor_scalar_mul(res, tot[0:1, 0:1], 1.0 / N)
    nc.sync.dma_start(out=out_d[:], in_=res)


def build_nc(cap_waits=True):
    nc = bass.Bass()
    a_d = nc.declare_dram_parameter("a", [K, N], F16, isOutput=False)
    b_d = nc.declare_dram_parameter("b", [K, N], F16, isOutput=False)
    ident_d = nc.declare_dram_parameter("ident", [128, 128], F16, isOutput=False)
    out_d = nc.declare_dram_parameter("out", [1, 1], F32, isOutput=True)
    with tile.TileContext(nc) as tc, ExitStack() as ctx:
        _emit_body(ctx, tc, a_d, b_d, ident_d, out_d)
    if cap_waits:
        # compile-path only: CoreSim can't handle the unregistered NoOps
        _cap_sync_waits(nc)
    return nc


_CACHE = {}

IDENT = np.eye(128, dtype=np.float16)


def _side(points):
    """Host prep of one set's fp16 split rows.

    points: [N, 3] fp32.  h = fp16(x), m = fp16(x - h); rh + rm ~= -|p|^2/2
    in two fp16 levels.  Paired products of the A/B row layouts below give
    hh' + hm' + mh' - r1/2 - r2/2 = D' per (n, m).
    """
    x = points.astype(np.float32).T  # [3, N]
    h = x.astype(np.float16)
    m = (x - h.astype(np.float32)).astype(np.float16)
    r = -0.5 * np.sum(x * x, axis=0)  # [N] fp32
    rh = r.astype(np.float16)
    rm = (r - rh.astype(np.float32)).astype(np.float16)
    ones = np.ones_like(rh)
    return h, m, rh, rm, ones


def make_in_maps(set1, set2):
    set1 = np.asarray(set1, dtype=np.float32)
    set2 = np.asarray(set2, dtype=np.float32)
    in_maps = []
    for b in range(B):
        h1, m1, rh1, rm1, on = _side(set1[b])
        h2, m2, rh2, rm2, _ = _side(set2[b])
        a = np.concatenate(
            [h1, h1, m1, rh1[None], rm1[None], on[None], on[None]], axis=0
        ).astype(np.float16)
        bb = np.concatenate(
            [h2, m2, h2, on[None], on[None], rh2[None], rm2[None]], axis=0
        ).astype(np.float16)
        in_maps.append({"a": np.ascontiguousarray(a),
                        "b": np.ascontiguousarray(bb),
                        "ident": IDENT})
    return in_maps


def kernel(set1, set2, _trace=False):
    if "nc" not in _CACHE:
        _CACHE["nc"] = build_nc()
    nc = _CACHE["nc"]
    r = run_bass_kernel_spmd(nc, make_in_maps(set1, set2),
                             core_ids=list(range(B)), trace=_trace)
    _CACHE["last_result"] = r
    return np.array([r.results[b]["out"][0, 0] for b in range(B)],
                    dtype=np.float32)
